# revision 36
# baseline (speedup 1.0000x reference)
"""AdaLN-modulated multi-head attention block on 8 TRN2 NeuronCores.

Shapes (hardcoded): B=8, T=1024, D=1024, H=16 heads, e=64 head dim.
Sharding: pure data-parallel - one batch element per core, weights
replicated, no collectives.

v2 design (vs baseline):
  - f32r moving/stationary weights: no f32->bf16 conversion pass.
  - per-head-pair pipeline: qk projection chunk -> stats -> affine+rope
    -> scores/exp/av per head, so ACT exp overlaps all PE work.
  - RoPE pair swap via DVE stream_shuffle (not PE matmul).
  - rope/affine elementwise all-bf16-SBUF for DVE 2x/4x perf modes.
  - psum evacuations distributed to Pool (gpsimd) / ACT.
  - bulk rearranged DMAs; gate + w_out loads deferred to mid-kernel.

Per-core pipeline ("T" suffix = feature-major [feature, token] layout):
  0. mod = silu(time) @ mod_w + mod_b (f32r moving); shift/scale cols.
  1. h = LN(x) (ACT apply) -> PE transpose -> fused affine evac -> hT
  2. v = h @ w_qkv[:, 2048:] (f32r moving, + ones col per head)
  3. per hp in 0..7:  qT/kT chunk (w stationary f32r),
     per-chunk LN stats (eseg matmul), affine+rope (DVE bf16),
     per head: scoresT = kT.T@qT; exp (ACT); oT = [v|1].T@exp;
     normalize via den-broadcast matmul + fused psum mul evac.
  4. y = (oT.T @ w_out) * gate (f32r moving w_out)
"""

import sys

try:
    import concourse  # noqa: F401  (provided by the environment, e.g. axon_site)
except ImportError:
    sys.path.append("/opt/trn_rl_repo")

import contextlib

import numpy as np

import concourse.bass as bass
import concourse.mybir as mybir
import concourse.tile as tile
from concourse import bacc
from concourse.bass_utils import run_bass_kernel_spmd

F32 = mybir.dt.float32
F32R = mybir.dt.float32r
BF16 = mybir.dt.bfloat16
AF = mybir.ActivationFunctionType
OP = mybir.AluOpType

B, T, D, TD = 8, 1024, 1024, 1024
H, E = 16, 64
P = 128
NT = T // P          # 8 token tiles
ND = D // P          # 8 feature tiles
EPS = 1e-6
N3 = 3 * D
SWAP_MASK = [i ^ 1 for i in range(32)]


def r(ap):
    """View an f32 AP as float32r for full-rate TensorE matmuls."""
    return ap.bitcast(F32R)


def build_nc(apply_qk_weight: bool):
    nc = bacc.Bacc("TRN2", target_bir_lowering=False, debug=False, num_devices=8)

    aps = {}
    aps["x"] = nc.dram_tensor("x", [T, D], F32, kind="ExternalInput").ap()
    aps["time"] = nc.dram_tensor("time", [TD], F32, kind="ExternalInput").ap()
    # weights arrive host-preconverted to bf16 (halves DMA, no on-chip casts)
    aps["mod_w"] = nc.dram_tensor("mod_w", [TD, N3], BF16, kind="ExternalInput").ap()
    aps["mod_b"] = nc.dram_tensor("mod_b", [N3], F32, kind="ExternalInput").ap()
    aps["w_qkv"] = nc.dram_tensor("w_qkv", [D, N3], BF16, kind="ExternalInput").ap()
    aps["w_out"] = nc.dram_tensor("w_out", [D, D], BF16, kind="ExternalInput").ap()
    # host-precomputed constants
    aps["cs_full"] = nc.dram_tensor("cs_full", [P, T], BF16, kind="ExternalInput").ap()
    aps["sn_full"] = nc.dram_tensor("sn_full", [P, T], BF16, kind="ExternalInput").ap()
    aps["eseg"] = nc.dram_tensor("eseg", [P, ND, 16], BF16, kind="ExternalInput").ap()
    aps["bseg2"] = nc.dram_tensor("bseg2", [2, P], BF16, kind="ExternalInput").ap()
    aps["ident"] = nc.dram_tensor("ident", [P, P], BF16, kind="ExternalInput").ap()
    aps["ones_row"] = nc.dram_tensor("ones_row", [1, P], BF16, kind="ExternalInput").ap()
    aps["wq_col"] = nc.dram_tensor("wq_col", [P, 1], F32, kind="ExternalInput").ap()
    aps["wk_col"] = nc.dram_tensor("wk_col", [P, 1], F32, kind="ExternalInput").ap()

    aps["out"] = nc.dram_tensor("out", [T, D], F32, kind="ExternalOutput").ap()

    with tile.TileContext(nc) as tc:
        _body(nc, tc, aps, apply_qk_weight)
    nc.finalize()
    return nc


def _body(nc, tc, aps, apply_qk_weight):
    x_e, time_e, modw_e = aps["x"], aps["time"], aps["mod_w"]
    modb_e, wqkv_e, wout_e = aps["mod_b"], aps["w_qkv"], aps["w_out"]
    out_e = aps["out"]

    ctx = contextlib.ExitStack()
    with ctx:
        consts = ctx.enter_context(tc.tile_pool(name="consts", bufs=1))
        big = ctx.enter_context(tc.tile_pool(name="big", bufs=1))
        wstr = ctx.enter_context(tc.tile_pool(name="wstr", bufs=1))
        temps = ctx.enter_context(tc.tile_pool(name="temps", bufs=2))
        small = ctx.enter_context(tc.tile_pool(name="small", bufs=1))
        psum = ctx.enter_context(tc.tile_pool(name="psum", bufs=2, space="PSUM"))

        # PSUM budget (8 banks of [128 x 2KB]):
        #   sc: 2 bufs [P,512]  - score halves
        #   po: 2 bufs [P,512]  - o accumulation (per head: tn0,tn1)
        #   pq: 2 bufs [P,512]  - q/k projection chains
        #   pm: 2 bufs [P,512]  - misc (stats, bcasts, mod, transposes, y)
        def ps(tag, shape, name, dtype=F32):
            return psum.tile(shape, dtype, tag=tag, bufs=2, name=name,
                             padded_shape=[P, 512])

        # ---- constants into SBUF -------------------------------------
        def cload(key, shape, dtype, name):
            t = consts.tile(shape, dtype, tag=name, name=name)
            nc.sync.dma_start(t[:], aps[key])
            return t

        cs_sb = cload("cs_full", [P, T], BF16, "cs_sb")
        sn_sb = cload("sn_full", [P, T], BF16, "sn_sb")
        eseg_sb = cload("eseg", [P, ND, 16], BF16, "eseg_sb")
        bseg2_sb = cload("bseg2", [2, P], BF16, "bseg2_sb")
        ident_sb = cload("ident", [P, P], BF16, "ident_sb")
        ones_sb = cload("ones_row", [1, P], BF16, "ones_sb")
        wq_sb = cload("wq_col", [P, 1], F32, "wq_sb")
        wk_sb = cload("wk_col", [P, 1], F32, "wk_sb")
        eps_sb = consts.tile([P, 1], F32, tag="eps_sb", name="eps_sb")
        nc.gpsimd.memset(eps_sb[:], EPS)

        # ---- big resident tensors ------------------------------------
        hT = big.tile([P, ND, T], BF16, tag="hT", name="hT")       # 16K/part
        qT = big.tile([P, ND, T], BF16, tag="qT", name="qT")       # 16K
        kT = big.tile([P, ND, T], BF16, tag="kT", name="kT")       # 16K
        v_sb = big.tile([P, NT, H, E + 1], BF16, tag="v", name="v_sb")  # 16.25K
        oTn = qT   # head rows of qT are dead once that head's scores ran

        # ==============================================================
        # DMA prefetch order: modw(shift+scale) -> x -> wqk -> wv ->
        # wout -> modw(gate).  Issue the early ones now.
        # ==============================================================
        # Shared [128, 8kc, 512] bf16 staging ring for mod_w / w_qkv-v /
        # w_out blocks (all the same shape; ring order = lifetime order).
        def wbig_tile(name):
            return wstr.tile([P, ND, 512], BF16, tag="wbig", bufs=3, name=name)

        # mod_w shift+scale halves first
        modw_sb = {}
        for g, n2 in ((0, 0), (0, 1), (1, 0), (1, 1)):
            mw = wbig_tile(f"modw{g}_{n2}")
            col0 = g * D + n2 * 512
            nc.sync.dma_start(
                mw[:], modw_e[:, col0:col0 + 512].rearrange(
                    "(kc p) j -> p kc j", p=P))
            modw_sb[(g, n2)] = mw

        # x tiles
        x_tiles = []
        for tt in range(NT):
            xt = temps.tile([P, D], F32, tag="xt", bufs=2, name=f"xt{tt}")
            nc.sync.dma_start(xt[:], x_e[tt * P:(tt + 1) * P, :])
            x_tiles.append(xt)

        # w_qkv q/k column blocks, one [128, 8kc, 128] tile per jc,
        # ordered by head pair: hp, 8+hp
        wqk_sb = {}
        for hp in range(ND):
            for jc in (hp, ND + hp):
                wt = wstr.tile([P, ND, P], BF16, tag="wqk", bufs=6,
                               name=f"wqk{jc}")
                nc.sync.dma_start(
                    wt[:], wqkv_e[:, jc * P:(jc + 1) * P].rearrange(
                        "(kc p) j -> p kc j", p=P))
                wqk_sb[jc] = wt

        # w_qkv v blocks: [128, 8kc, 512] per nv (wbig ring, after mod)
        wv_sb = []
        for nv in range(2):
            wv = wbig_tile(f"wv{nv}")
            nc.sync.dma_start(
                wv[:], wqkv_e[:, 2048 + nv * 512:2048 + (nv + 1) * 512]
                .rearrange("(kc p) j -> p kc j", p=P))
            wv_sb.append(wv)

        # ==============================================================
        # Stage 0: mod = silu(time) @ mod_w + mod_b (shift/scale now)
        # ==============================================================
        t8 = small.tile([P, TD // P], F32, tag="t8", name="t8")
        nc.sync.dma_start(t8[:], time_e.rearrange("(o p) -> p o", p=P))
        silu8 = small.tile([P, TD // P], BF16, tag="silu8", name="silu8")
        nc.scalar.activation(silu8[:], t8[:], AF.Silu)

        def mod_group(g, mw_pair):
            """Compute mod group g -> [1, D] bf16 row (bias added)."""
            mrowf = temps.tile([1, D], F32, tag="rbc", bufs=1, name=f"mrowf{g}")
            nc.sync.dma_start(mrowf[:], modb_e[None, g * D:(g + 1) * D])
            mrow = small.tile([1, D], BF16, tag=f"mrow{g}", bufs=1,
                              name=f"mrow{g}")
            for n2 in range(2):
                mw = mw_pair[n2]
                mp = ps("pm", [1, 512], f"modp{g}_{n2}")
                for kc in range(TD // P):
                    nc.tensor.matmul(mp[:], silu8[:, kc:kc + 1], mw[:, kc, :],
                                     start=(kc == 0), stop=(kc == TD // P - 1))
                sl = slice(n2 * 512, (n2 + 1) * 512)
                nc.vector.tensor_add(mrow[:, sl], mrowf[:, sl], mp[:])
            return mrow

        def mod_to_cols(g, mrow, plus1):
            """Transpose a [1, D] mod row into per-feature cols [P, ND]."""
            if plus1:
                nc.vector.tensor_scalar_add(mrow[:], mrow[:], 1.0)
            col = consts.tile([P, ND], F32, tag=f"col{g}", name=f"col{g}")
            cp = psum.tile([P, 2 * ND], BF16, tag="pm", bufs=2,
                           name=f"colp{g}", padded_shape=[P, 512])
            for dc in range(ND):
                nc.tensor.transpose(cp[:, 2 * dc:2 * dc + 1],
                                    mrow[:, dc * P:(dc + 1) * P],
                                    ident_sb[0:1, 0:1])
            nc.vector.tensor_copy(col[:], cp[:].rearrange(
                "p (d two) -> p d two", two=2)[:, :, 0])
            return col

        shcol = mod_to_cols(0, mod_group(0, (modw_sb[(0, 0)], modw_sb[(0, 1)])),
                            False)
        sc1col = mod_to_cols(1, mod_group(1, (modw_sb[(1, 0)], modw_sb[(1, 1)])),
                             True)

        # ==============================================================
        # Stage 1: h = LN(x)*(scale+1)+shift ; hT = h.T
        # ==============================================================
        for tt in range(NT):
            xt = x_tiles[tt]
            st = small.tile([P, 2, 6], F32, tag="bnst", bufs=2, name=f"st{tt}")
            nc.vector.bn_stats(st[:, 0, :], xt[:, 0:512])
            nc.vector.bn_stats(st[:, 1, :], xt[:, 512:1024])
            mv = small.tile([P, 2], F32, tag="bnmv", bufs=2, name=f"mv{tt}")
            nc.vector.bn_aggr(mv[:], st[:])
            sd = small.tile([P, 1], F32, tag="sd", bufs=2, name=f"sd{tt}")
            nc.scalar.activation(sd[:], mv[:, 1:2], AF.Sqrt, bias=eps_sb[:])
            rstd = small.tile([P, 1], F32, tag="rstd", bufs=2, name=f"rstd{tt}")
            nc.vector.reciprocal(rstd[:], sd[:])
            nmr = small.tile([P, 1], F32, tag="nmr", bufs=2, name=f"nmr{tt}")
            nc.vector.tensor_scalar(nmr[:], mv[:, 0:1], rstd[:], -1.0,
                                    OP.mult, OP.mult)
            xn = temps.tile([P, D], BF16, tag="xnb", bufs=2, name=f"xn{tt}")
            nc.scalar.activation(xn[:], xt[:], AF.Identity, bias=nmr[:],
                                 scale=rstd[:])
            # 4 transposes per [P,512] bf16 psum buf, fused affine evac
            for half in range(2):
                tp = psum.tile([P, 512], BF16, tag="pq", bufs=2,
                               name=f"tr{tt}_{half}", padded_shape=[P, 512])
                for q in range(4):
                    dc = half * 4 + q
                    nc.tensor.transpose(tp[:, q * P:(q + 1) * P],
                                        xn[:, dc * P:(dc + 1) * P], ident_sb[:])
                for q in range(4):
                    dc = half * 4 + q
                    if dc % 2 == 0:
                        nc.vector.tensor_scalar(
                            hT[:, dc, tt * P:(tt + 1) * P],
                            tp[:, q * P:(q + 1) * P],
                            sc1col[:, dc:dc + 1], shcol[:, dc:dc + 1],
                            OP.mult, OP.add)
                    else:
                        nc.scalar.activation(
                            hT[:, dc, tt * P:(tt + 1) * P],
                            tp[:, q * P:(q + 1) * P], AF.Identity,
                            bias=shcol[:, dc:dc + 1],
                            scale=sc1col[:, dc:dc + 1])

        # ==============================================================
        # Stage 2a: v = h @ w_qkv[:, 2048:] (+ ones col per head)
        # ==============================================================
        nc.gpsimd.memset(v_sb[:, :, :, E], 1.0)

        def v_block(tt):
            vps = [ps("po", [P, 512], f"vp{tt}_{nv}") for nv in range(2)]
            for kc in range(ND):
                for nv in range(2):
                    nc.tensor.matmul(vps[nv][:], hT[:, kc, tt * P:(tt + 1) * P],
                                     wv_sb[nv][:, kc, :],
                                     start=(kc == 0), stop=(kc == ND - 1))
            for nv in range(2):
                nc.scalar.copy(
                    v_sb[:, tt, nv * 8:(nv + 1) * 8, 0:E],
                    vps[nv][:].rearrange("p (h e) -> p h e", e=E))

        # ==============================================================
        # Per-head-pair building blocks
        # ==============================================================
        def qk_chunk(jc):
            """Project w_qkv col block jc against hT -> qT/kT chunk."""
            dst = qT if jc < ND else kT
            jd = jc % ND
            wt = wqk_sb[jc]
            qps = [ps("pq", [P, 512], f"qp{jc}_{tn}") for tn in range(2)]
            for kc in range(ND):
                for tn in range(2):
                    nc.tensor.matmul(qps[tn][:], wt[:, kc, :],
                                     hT[:, kc, tn * 512:(tn + 1) * 512],
                                     start=(kc == 0), stop=(kc == ND - 1))
            for tn in range(2):
                nc.vector.tensor_copy(dst[:, jd, tn * 512:(tn + 1) * 512],
                                      qps[tn][:])

        def stats_ab(which, jt):
            """Per-chunk LN stats -> A2 (rstd) / B2 (-mean*rstd) [2, T] bf16.

            stats psum layout in one [P,512] buf per tn:
              rows 0:2 = sum, rows 64:66 = sumsq  (tile_position rule:
              out base partition must be 0/32/64/96 for tiny matmuls)
            """
            src_t = qT if which == 0 else kT
            sq = temps.tile([P, T], BF16, tag="sqt", bufs=1, name=f"sq{which}_{jt}")
            nc.scalar.activation(sq[:], src_t[:, jt, :], AF.Square)
            A2 = small.tile([2, T], BF16, tag="Asb", bufs=2, name=f"A{which}_{jt}")
            B2 = small.tile([2, T], BF16, tag="Bsb", bufs=2, name=f"B{which}_{jt}")
            es = eseg_sb[:, jt, 2 * jt:2 * jt + 2]
            for tn in range(2):
                sl = slice(tn * 512, (tn + 1) * 512)
                stp = ps("pm", [P, 512], f"st{which}_{jt}_{tn}")
                nc.tensor.matmul(stp[0:2, :], es, src_t[:, jt, sl])
                nc.tensor.matmul(stp[64:66, :], es, sq[:, sl])

                def scr2(nm):
                    return small.tile([2, 512], F32, tag="scr2", bufs=4,
                                      name=f"{nm}_{which}_{jt}_{tn}")

                # mu2 = (sum/E)^2 ; var = ssq/E - mu2 ; A = 1/sqrt(var+eps)
                t2 = scr2("t2")
                nc.scalar.activation(t2[:], stp[0:2, :], AF.Square,
                                     scale=1.0 / E)
                u2 = scr2("u2")
                nc.vector.scalar_tensor_tensor(u2[:], stp[64:66, :], 1.0 / E,
                                               t2[:], OP.mult, OP.subtract)
                sd2 = scr2("sd")
                nc.scalar.activation(sd2[:], u2[:], AF.Sqrt, bias=eps_sb[0:2])
                with nc.allow_low_precision(reason="bf16 LN rstd"):
                    nc.vector.reciprocal(A2[:, sl], sd2[:])
                muf = scr2("mu")
                nc.vector.tensor_scalar_mul(muf[:], stp[0:2, :], 1.0 / E)
                nc.vector.scalar_tensor_tensor(B2[:, sl], muf[:], -1.0,
                                               A2[:, sl], OP.mult, OP.mult)
            return A2, B2

        def bcast_ab(which, jt, A2, B2):
            """Broadcast A2/B2 [2,T] -> [128,T] bf16 SBUF via PE + Pool."""
            outs = []
            for nm, src in (("A", A2), ("B", B2)):
                bc = temps.tile([P, T], BF16, tag=f"bc{nm}", bufs=2,
                                name=f"bc{nm}{which}_{jt}")
                for tn in range(2):
                    sl = slice(tn * 512, (tn + 1) * 512)
                    bp = ps("pm", [P, 512], f"bp{nm}{which}_{jt}_{tn}")
                    nc.tensor.matmul(bp[:], bseg2_sb[:], src[:, sl])
                    nc.vector.tensor_copy(bc[:, sl], bp[:])
                outs.append(bc)
            return outs

        def affine_rope(which, jt, bcA, bcB):
            """src = rope(src*bcA + bcB) in place, all bf16 SBUF on DVE."""
            src = (qT if which == 0 else kT)[:, jt, :]
            wcol = wq_sb if which == 0 else wk_sb
            t2 = temps.tile([P, T], BF16, tag="ropet", bufs=2,
                            name=f"t2r{which}_{jt}")
            nc.vector.tensor_mul(t2[:], src, bcA[:])
            nc.vector.tensor_add(t2[:], t2[:], bcB[:])
            if apply_qk_weight:
                nc.vector.tensor_scalar_mul(t2[:], t2[:], wcol[:])
            shf = temps.tile([P, T], BF16, tag="ropes", bufs=2,
                             name=f"shf{which}_{jt}")
            nc.vector.stream_shuffle(shf[:], t2[:], SWAP_MASK)
            nc.vector.tensor_mul(src, t2[:], cs_sb[:])
            nc.vector.tensor_mul(shf[:], shf[:], sn_sb[:])
            nc.vector.tensor_add(src, src, shf[:])

        def head_attention(h):
            """scores -> exp -> o accumulation -> normalized oTn rows."""
            jc = h // 2
            p0 = (h % 2) * E
            ops = [ps("po", [E + 1, 512], f"o{h}_{tn}") for tn in range(2)]

            def av(tk, ex):
                for tn in range(2):
                    sl = slice(tn * 512, (tn + 1) * 512)
                    nc.tensor.matmul(ops[tn][:], v_sb[:, tk, h, :], ex[:, sl],
                                     start=(tk == 0), stop=(tk == NT - 1))

            # 1-deep software pipeline: exp(tk) overlaps av(tk-1)+sc(tk+1)
            prev = None
            for tk in range(NT):
                ex = temps.tile([P, T], BF16, tag="exp", bufs=3,
                                name=f"ex{h}_{tk}")
                for tn in range(2):
                    sl = slice(tn * 512, (tn + 1) * 512)
                    sc = ps("sc", [P, 512], f"sc{h}_{tk}_{tn}")
                    nc.tensor.matmul(sc[:],
                                     kT[p0:p0 + E, jc, tk * P:(tk + 1) * P],
                                     qT[p0:p0 + E, jc, sl])
                    nc.scalar.activation(ex[:, sl], sc[:], AF.Exp, scale=0.125)
                if prev is not None:
                    av(*prev)
                prev = (tk, ex)
            av(*prev)
            # denominator row -> reciprocal -> broadcast to [E, 512] -> fused
            # normalize evac into oTn rows
            rcp = small.tile([1, T], BF16, tag="rcp", bufs=2, name=f"rcp{h}")
            with nc.allow_low_precision(reason="bf16 softmax denominators"):
                for tn in range(2):
                    sl = slice(tn * 512, (tn + 1) * 512)
                    nc.vector.reciprocal(rcp[:, sl], ops[tn][E:E + 1, :])
            brs = temps.tile([E, T], BF16, tag="brs", bufs=2, name=f"brs{h}")
            for tn in range(2):
                sl = slice(tn * 512, (tn + 1) * 512)
                br = ps("pm", [E, 512], f"br{h}_{tn}")
                nc.tensor.matmul(br[:], ones_sb[0:1, 0:E], rcp[:, sl])
                nc.vector.tensor_copy(brs[:, sl], br[:])
                nc.vector.tensor_mul(oTn[p0:p0 + E, jc, sl], ops[tn][0:E, :],
                                     brs[:, sl])

        # ==============================================================
        # Pipeline: qk chunks run 2 hps ahead; v blocks interleaved
        # ==============================================================
        qk_chunk(0)
        qk_chunk(ND)
        qk_chunk(1)
        qk_chunk(ND + 1)
        for tt in range(NT):
            v_block(tt)

        for hp in range(ND):
            if hp + 2 < ND:
                qk_chunk(hp + 2)
                qk_chunk(ND + hp + 2)
            A2q, B2q = stats_ab(0, hp)
            A2k, B2k = stats_ab(1, hp)
            bcAq, bcBq = bcast_ab(0, hp, A2q, B2q)
            affine_rope(0, hp, bcAq, bcBq)
            bcAk, bcBk = bcast_ab(1, hp, A2k, B2k)
            affine_rope(1, hp, bcAk, bcBk)
            head_attention(2 * hp)
            head_attention(2 * hp + 1)
            if hp == 0:
                # mid-kernel deferred DMAs: mod_w gate block (wbig ring)
                for n2 in range(2):
                    mw = wbig_tile(f"modwg_{n2}")
                    col0 = 2 * D + n2 * 512
                    nc.sync.dma_start(
                        mw[:], modw_e[:, col0:col0 + 512].rearrange(
                            "(kc p) j -> p kc j", p=P))
                    modw_sb[(2, n2)] = mw
            if hp == 1:
                # gate row -> broadcast [P, D] bf16
                growb = mod_group(2, (modw_sb[(2, 0)], modw_sb[(2, 1)]))
                gateB = consts.tile([P, D], BF16, tag="gateB", name="gateB")
                for n2 in range(2):
                    sl = slice(n2 * 512, (n2 + 1) * 512)
                    bp = ps("pm", [P, 512], f"gbc{n2}")
                    nc.tensor.matmul(bp[:], ones_sb[:], growb[:, sl])
                    nc.vector.tensor_copy(gateB[:, sl], bp[:])
            if hp == 2:
                # w_out halves (wbig ring)
                wof = [wbig_tile(f"wof{tn}") for tn in range(2)]
                for tn in range(2):
                    nc.sync.dma_start(
                        wof[tn][:],
                        wout_e[:, tn * 512:(tn + 1) * 512].rearrange(
                            "(kc p) j -> p kc j", p=P))

        # ==============================================================
        # Stage 5: y = (oTn.T @ w_out) * gate
        # ==============================================================
        for tt in range(NT):
            y_sb = temps.tile([P, D], F32, tag="ysb", bufs=2, name=f"y{tt}")
            yps = [ps("pq", [P, 512], f"yp{tt}_{tn}") for tn in range(2)]
            for kc in range(ND):
                for tn in range(2):
                    nc.tensor.matmul(yps[tn][:], oTn[:, kc, tt * P:(tt + 1) * P],
                                     wof[tn][:, kc, :],
                                     start=(kc == 0), stop=(kc == ND - 1))
            for tn in range(2):
                sl = slice(tn * 512, (tn + 1) * 512)
                nc.vector.tensor_mul(y_sb[:, sl], yps[tn][:], gateB[:, sl])
            nc.sync.dma_start(out_e[tt * P:(tt + 1) * P, :], y_sb[:])


# =====================================================================
# Host side
# =====================================================================
_NC_CACHE = {}


def _get_nc(apply_qk_weight: bool):
    key = bool(apply_qk_weight)
    if key not in _NC_CACHE:
        _NC_CACHE[key] = build_nc(key)
    return _NC_CACHE[key]


def _make_consts(position, q_norm_w, k_norm_w):
    cs = np.ones((P, T), np.float32)
    sn = np.zeros((P, T), np.float32)
    cos = position[:, :, 0].T.astype(np.float32)   # [16, T]
    sin = position[:, :, 1].T.astype(np.float32)
    for half in (0, 64):
        for rr in range(32):
            j = rr // 2
            cs[half + rr, :] = cos[j]
            sn[half + rr, :] = sin[j] if (rr % 2 == 1) else -sin[j]
    eseg = np.zeros((P, ND, 16), np.float32)
    for t in range(ND):
        for p in range(P):
            m = 2 * t + p // E
            eseg[p, t, m] = 1.0
    bseg2 = np.zeros((2, P), np.float32)
    bseg2[0, 0:E] = 1.0
    bseg2[1, E:P] = 1.0
    import ml_dtypes  # noqa: deferred import keeps numpy-only callers fast
    return dict(
        cs_full=cs.astype(ml_dtypes.bfloat16), sn_full=sn.astype(ml_dtypes.bfloat16),
        eseg=eseg.astype(ml_dtypes.bfloat16),
        bseg2=bseg2.astype(ml_dtypes.bfloat16),
        ident=np.eye(P, dtype=np.float32).astype(ml_dtypes.bfloat16),
        ones_row=np.ones((1, P), np.float32).astype(ml_dtypes.bfloat16),
        wq_col=np.tile(q_norm_w.astype(np.float32), 2).reshape(P, 1),
        wk_col=np.tile(k_norm_w.astype(np.float32), 2).reshape(P, 1),
    )


def _prep_weights(mod_w, w_qkv, w_out):
    import ml_dtypes
    return dict(
        mod_w=np.ascontiguousarray(np.asarray(mod_w, np.float32)
                                   .astype(ml_dtypes.bfloat16)),
        w_qkv=np.ascontiguousarray(np.asarray(w_qkv, np.float32)
                                   .astype(ml_dtypes.bfloat16)),
        w_out=np.ascontiguousarray(np.asarray(w_out, np.float32)
                                   .astype(ml_dtypes.bfloat16)),
    )


def kernel(x, time, position, mod_w, mod_b, w_qkv, w_out, q_norm_w, k_norm_w):
    x = np.ascontiguousarray(np.asarray(x, dtype=np.float32))
    time = np.ascontiguousarray(np.asarray(time, dtype=np.float32))
    position = np.asarray(position, dtype=np.float32)
    mod_b = np.ascontiguousarray(np.asarray(mod_b, dtype=np.float32))
    q_norm_w = np.asarray(q_norm_w, dtype=np.float32)
    k_norm_w = np.asarray(k_norm_w, dtype=np.float32)
    wts = _prep_weights(mod_w, w_qkv, w_out)

    apply_w = not (np.all(q_norm_w == 1.0) and np.all(k_norm_w == 1.0))
    nc = _get_nc(apply_w)
    consts = _make_consts(position, q_norm_w, k_norm_w)

    in_maps = [
        dict(x=x[b], time=time[b].reshape(TD), mod_b=mod_b, **wts, **consts)
        for b in range(B)
    ]
    res = run_bass_kernel_spmd(nc, in_maps, core_ids=list(range(B)))
    out = np.stack([res.results[b]["out"] for b in range(B)], axis=0)
    return out.astype(np.float32)


if __name__ == "__main__":
    nc = build_nc(False)
    print("graph built ok")


# revision 43
# speedup vs baseline: 1.4946x; 1.4946x over previous
"""AdaLN-modulated multi-head attention block on 8 TRN2 NeuronCores.

Shapes (hardcoded): B=8, T=1024, D=1024, H=16 heads, e=64 head dim.
Sharding: pure data-parallel - one batch element per core, weights
replicated, no collectives.

v2 design (vs baseline):
  - f32r moving/stationary weights: no f32->bf16 conversion pass.
  - per-head-pair pipeline: qk projection chunk -> stats -> affine+rope
    -> scores/exp/av per head, so ACT exp overlaps all PE work.
  - RoPE pair swap via DVE stream_shuffle (not PE matmul).
  - rope/affine elementwise all-bf16-SBUF for DVE 2x/4x perf modes.
  - psum evacuations distributed to Pool (gpsimd) / ACT.
  - bulk rearranged DMAs; gate + w_out loads deferred to mid-kernel.

Per-core pipeline ("T" suffix = feature-major [feature, token] layout):
  0. mod = silu(time) @ mod_w + mod_b (f32r moving); shift/scale cols.
  1. h = LN(x) (ACT apply) -> PE transpose -> fused affine evac -> hT
  2. v = h @ w_qkv[:, 2048:] (f32r moving, + ones col per head)
  3. per hp in 0..7:  qT/kT chunk (w stationary f32r),
     per-chunk LN stats (eseg matmul), affine+rope (DVE bf16),
     per head: scoresT = kT.T@qT; exp (ACT); oT = [v|1].T@exp;
     normalize via den-broadcast matmul + fused psum mul evac.
  4. y = (oT.T @ w_out) * gate (f32r moving w_out)
"""

import sys

try:
    import concourse  # noqa: F401  (provided by the environment, e.g. axon_site)
except ImportError:
    sys.path.append("/opt/trn_rl_repo")

import contextlib

import numpy as np

import concourse.bass as bass
import concourse.mybir as mybir
import concourse.tile as tile
from concourse import bacc
from concourse.bass_utils import run_bass_kernel_spmd

F32 = mybir.dt.float32
F32R = mybir.dt.float32r
BF16 = mybir.dt.bfloat16
AF = mybir.ActivationFunctionType
OP = mybir.AluOpType

B, T, D, TD = 8, 1024, 1024, 1024
H, E = 16, 64
P = 128
NT = T // P          # 8 token tiles
ND = D // P          # 8 feature tiles
EPS = 1e-6
N3 = 3 * D
SWAP_MASK = [i ^ 1 for i in range(32)]


def r(ap):
    """View an f32 AP as float32r for full-rate TensorE matmuls."""
    return ap.bitcast(F32R)


def build_nc(apply_qk_weight: bool):
    nc = bacc.Bacc("TRN2", target_bir_lowering=False, debug=False, num_devices=8)

    aps = {}
    aps["x"] = nc.dram_tensor("x", [T, D], F32, kind="ExternalInput").ap()
    aps["time"] = nc.dram_tensor("time", [TD], F32, kind="ExternalInput").ap()
    # weights arrive host-preconverted to bf16 (halves DMA, no on-chip casts)
    aps["mod_w"] = nc.dram_tensor("mod_w", [TD, N3], BF16, kind="ExternalInput").ap()
    aps["mod_b"] = nc.dram_tensor("mod_b", [N3], F32, kind="ExternalInput").ap()
    aps["w_qkv"] = nc.dram_tensor("w_qkv", [D, N3], BF16, kind="ExternalInput").ap()
    aps["w_out"] = nc.dram_tensor("w_out", [D, D], BF16, kind="ExternalInput").ap()
    # host-precomputed constants
    aps["cs_full"] = nc.dram_tensor("cs_full", [P, T], BF16, kind="ExternalInput").ap()
    aps["sn_full"] = nc.dram_tensor("sn_full", [P, T], BF16, kind="ExternalInput").ap()
    aps["eseg"] = nc.dram_tensor("eseg", [P, ND, 16], BF16, kind="ExternalInput").ap()
    aps["bsegj"] = nc.dram_tensor("bsegj", [8, 4, P], BF16, kind="ExternalInput").ap()
    aps["bseg16"] = nc.dram_tensor("bseg16", [16, ND, P], BF16,
                                   kind="ExternalInput").ap()
    aps["ident"] = nc.dram_tensor("ident", [P, P], BF16, kind="ExternalInput").ap()
    aps["ones_row"] = nc.dram_tensor("ones_row", [1, P], BF16, kind="ExternalInput").ap()
    aps["wq_col"] = nc.dram_tensor("wq_col", [P, 1], F32, kind="ExternalInput").ap()
    aps["wk_col"] = nc.dram_tensor("wk_col", [P, 1], F32, kind="ExternalInput").ap()

    aps["out"] = nc.dram_tensor("out", [T, D], F32, kind="ExternalOutput").ap()

    with tile.TileContext(nc) as tc:
        _body(nc, tc, aps, apply_qk_weight)
    nc.finalize()
    return nc


def _body(nc, tc, aps, apply_qk_weight):
    x_e, time_e, modw_e = aps["x"], aps["time"], aps["mod_w"]
    modb_e, wqkv_e, wout_e = aps["mod_b"], aps["w_qkv"], aps["w_out"]
    out_e = aps["out"]

    ctx = contextlib.ExitStack()
    with ctx:
        consts = ctx.enter_context(tc.tile_pool(name="consts", bufs=1))
        big = ctx.enter_context(tc.tile_pool(name="big", bufs=1))
        wstr = ctx.enter_context(tc.tile_pool(name="wstr", bufs=1))
        temps = ctx.enter_context(tc.tile_pool(name="temps", bufs=2))
        small = ctx.enter_context(tc.tile_pool(name="small", bufs=1))
        psum = ctx.enter_context(tc.tile_pool(name="psum", bufs=2, space="PSUM"))

        # PSUM budget (8 banks of [128 x 2KB]):
        #   sc: 2 bufs [P,512]  - score halves
        #   po: 2 bufs [P,512]  - o accumulation (per head: tn0,tn1)
        #   pq: 2 bufs [P,512]  - q/k projection chains
        #   pm: 2 bufs [P,512]  - misc (stats, bcasts, mod, transposes, y)
        def ps(tag, shape, name, dtype=F32):
            return psum.tile(shape, dtype, tag=tag, bufs=2, name=name,
                             padded_shape=[P, 512])

        # ---- constants into SBUF -------------------------------------
        def cload(key, shape, dtype, name):
            t = consts.tile(shape, dtype, tag=name, name=name)
            nc.sync.dma_start(t[:], aps[key])
            return t

        cs_sb = cload("cs_full", [P, T], BF16, "cs_sb")
        sn_sb = cload("sn_full", [P, T], BF16, "sn_sb")
        eseg_sb = cload("eseg", [P, ND, 16], BF16, "eseg_sb")
        bsegj_all = cload("bsegj", [8, 4, P], BF16, "bsegj_sb")
        bsegj_sb = [bsegj_all[:, j, :] for j in range(4)]
        bseg16_sb = cload("bseg16", [16, ND, P], BF16, "bseg16_sb")
        ident_sb = cload("ident", [P, P], BF16, "ident_sb")
        ones_sb = cload("ones_row", [1, P], BF16, "ones_sb")
        wq_sb = cload("wq_col", [P, 1], F32, "wq_sb")
        wk_sb = cload("wk_col", [P, 1], F32, "wk_sb")
        eps_sb = consts.tile([P, 1], F32, tag="eps_sb", name="eps_sb")
        nc.gpsimd.memset(eps_sb[:], EPS)

        # ---- big resident tensors ------------------------------------
        hT = big.tile([P, ND, T], BF16, tag="hT", name="hT")       # 16K/part
        qT = big.tile([P, ND, T], BF16, tag="qT", name="qT")       # 16K
        kT = big.tile([P, ND, T], BF16, tag="kT", name="kT")       # 16K
        v_sb = big.tile([P, NT, H, E + 16], BF16, tag="v", name="v_sb")  # 20K
        oTn = qT   # head rows of qT are dead once that head's scores ran

        # ==============================================================
        # DMA prefetch order: modw(shift+scale) -> x -> wqk -> wv ->
        # wout -> modw(gate).  Issue the early ones now.
        # ==============================================================
        # Shared [128, 8kc, 512] bf16 staging ring for mod_w / w_qkv-v /
        # w_out blocks (all the same shape; ring order = lifetime order).
        def wbig_tile(name):
            return wstr.tile([P, ND, 512], BF16, tag="wbig", bufs=3, name=name)

        # mod_w shift+scale halves first
        modw_sb = {}
        for g, n2 in ((0, 0), (0, 1), (1, 0), (1, 1)):
            mw = wbig_tile(f"modw{g}_{n2}")
            col0 = g * D + n2 * 512
            nc.sync.dma_start(
                mw[:], modw_e[:, col0:col0 + 512].rearrange(
                    "(kc p) j -> p kc j", p=P))
            modw_sb[(g, n2)] = mw

        # x tiles
        x_tiles = []
        for tt in range(NT):
            xt = temps.tile([P, D], F32, tag="xt", bufs=2, name=f"xt{tt}")
            nc.sync.dma_start(xt[:], x_e[tt * P:(tt + 1) * P, :])
            x_tiles.append(xt)

        # w_qkv q/k column blocks, one [128, 8kc, 128] tile per jc,
        # ordered by head pair: hp, 8+hp
        wqk_sb = {}
        for hp in range(ND):
            for jc in (hp, ND + hp):
                wt = wstr.tile([P, ND, P], BF16, tag="wqk", bufs=6,
                               name=f"wqk{jc}")
                nc.sync.dma_start(
                    wt[:], wqkv_e[:, jc * P:(jc + 1) * P].rearrange(
                        "(kc p) j -> p kc j", p=P))
                wqk_sb[jc] = wt

        # w_qkv v blocks: [128, 8kc, 512] per nv (wbig ring, after mod)
        wv_sb = []
        for nv in range(2):
            wv = wbig_tile(f"wv{nv}")
            nc.sync.dma_start(
                wv[:], wqkv_e[:, 2048 + nv * 512:2048 + (nv + 1) * 512]
                .rearrange("(kc p) j -> p kc j", p=P))
            wv_sb.append(wv)

        # ==============================================================
        # Stage 0: mod = silu(time) @ mod_w + mod_b (shift/scale now)
        # ==============================================================
        t8 = small.tile([P, TD // P], F32, tag="t8", name="t8")
        nc.sync.dma_start(t8[:], time_e.rearrange("(o p) -> p o", p=P))
        silu8 = small.tile([P, TD // P], BF16, tag="silu8", name="silu8")
        nc.scalar.activation(silu8[:], t8[:], AF.Silu)

        def mod_group(g, mw_pair):
            """Compute mod group g -> [1, D] bf16 row (bias added)."""
            mrowf = temps.tile([1, D], F32, tag="rbc", bufs=1, name=f"mrowf{g}")
            nc.sync.dma_start(mrowf[:], modb_e[None, g * D:(g + 1) * D])
            mrow = small.tile([1, D], BF16, tag=f"mrow{g}", bufs=1,
                              name=f"mrow{g}")
            for n2 in range(2):
                mw = mw_pair[n2]
                mp = ps("pm", [1, 512], f"modp{g}_{n2}")
                for kc in range(TD // P):
                    nc.tensor.matmul(mp[:], silu8[:, kc:kc + 1], mw[:, kc, :],
                                     start=(kc == 0), stop=(kc == TD // P - 1))
                sl = slice(n2 * 512, (n2 + 1) * 512)
                nc.vector.tensor_add(mrow[:, sl], mrowf[:, sl], mp[:])
            return mrow

        def mod_to_cols(g, mrow, plus1):
            """Transpose a [1, D] mod row into per-feature cols [P, ND]."""
            if plus1:
                nc.vector.tensor_scalar_add(mrow[:], mrow[:], 1.0)
            col = consts.tile([P, ND], F32, tag=f"col{g}", name=f"col{g}")
            cp = psum.tile([P, 2 * ND], BF16, tag="pm", bufs=2,
                           name=f"colp{g}", padded_shape=[P, 512])
            for dc in range(ND):
                nc.tensor.transpose(cp[:, 2 * dc:2 * dc + 1],
                                    mrow[:, dc * P:(dc + 1) * P],
                                    ident_sb[0:1, 0:1])
            nc.vector.tensor_copy(col[:], cp[:].rearrange(
                "p (d two) -> p d two", two=2)[:, :, 0])
            return col

        shcol = mod_to_cols(0, mod_group(0, (modw_sb[(0, 0)], modw_sb[(0, 1)])),
                            False)
        sc1col = mod_to_cols(1, mod_group(1, (modw_sb[(1, 0)], modw_sb[(1, 1)])),
                             True)

        # ==============================================================
        # Stage 1: h = LN(x)*(scale+1)+shift ; hT = h.T
        # ==============================================================
        for tt in range(NT):
            xt = x_tiles[tt]
            st = small.tile([P, 2, 6], F32, tag="bnst", bufs=2, name=f"st{tt}")
            nc.vector.bn_stats(st[:, 0, :], xt[:, 0:512])
            nc.vector.bn_stats(st[:, 1, :], xt[:, 512:1024])
            mv = small.tile([P, 2], F32, tag="bnmv", bufs=2, name=f"mv{tt}")
            nc.vector.bn_aggr(mv[:], st[:])
            sd = small.tile([P, 1], F32, tag="sd", bufs=2, name=f"sd{tt}")
            nc.scalar.activation(sd[:], mv[:, 1:2], AF.Sqrt, bias=eps_sb[:])
            rstd = small.tile([P, 1], F32, tag="rstd", bufs=2, name=f"rstd{tt}")
            nc.vector.reciprocal(rstd[:], sd[:])
            nmr = small.tile([P, 1], F32, tag="nmr", bufs=2, name=f"nmr{tt}")
            nc.vector.tensor_scalar(nmr[:], mv[:, 0:1], rstd[:], -1.0,
                                    OP.mult, OP.mult)
            xn = temps.tile([P, D], BF16, tag="xnb", bufs=2, name=f"xn{tt}")
            nc.scalar.activation(xn[:], xt[:], AF.Identity, bias=nmr[:],
                                 scale=rstd[:])
            # 4 transposes per [P,512] bf16 psum buf, fused affine evac
            for half in range(2):
                tp = psum.tile([P, 512], BF16, tag="pq", bufs=2,
                               name=f"tr{tt}_{half}", padded_shape=[P, 512])
                for q in range(4):
                    dc = half * 4 + q
                    nc.tensor.transpose(tp[:, q * P:(q + 1) * P],
                                        xn[:, dc * P:(dc + 1) * P], ident_sb[:])
                for q in range(4):
                    dc = half * 4 + q
                    if dc % 2 == 0:
                        nc.vector.tensor_scalar(
                            hT[:, dc, tt * P:(tt + 1) * P],
                            tp[:, q * P:(q + 1) * P],
                            sc1col[:, dc:dc + 1], shcol[:, dc:dc + 1],
                            OP.mult, OP.add)
                    else:
                        nc.scalar.activation(
                            hT[:, dc, tt * P:(tt + 1) * P],
                            tp[:, q * P:(q + 1) * P], AF.Identity,
                            bias=shcol[:, dc:dc + 1],
                            scale=sc1col[:, dc:dc + 1])

        # ==============================================================
        # Stage 2a: v = h @ w_qkv[:, 2048:] (+ ones col per head)
        # ==============================================================
        nc.gpsimd.memset(v_sb[:, :, :, E:E + 16], 0.0)
        for h in range(H):
            nc.gpsimd.memset(v_sb[:, :, h, E + h], 1.0)

        def v_block(tt):
            vps = [ps("po", [P, 512], f"vp{tt}_{nv}") for nv in range(2)]
            for kc in range(ND):
                for nv in range(2):
                    nc.tensor.matmul(vps[nv][:], hT[:, kc, tt * P:(tt + 1) * P],
                                     wv_sb[nv][:, kc, :],
                                     start=(kc == 0), stop=(kc == ND - 1))
            for nv in range(2):
                nc.scalar.copy(
                    v_sb[:, tt, nv * 8:(nv + 1) * 8, 0:E],
                    vps[nv][:].rearrange("p (h e) -> p h e", e=E))

        # ==============================================================
        # Building blocks
        # ==============================================================
        def qk_chunk(jc):
            """Project w_qkv col block jc against hT -> qT/kT chunk."""
            dst = qT if jc < ND else kT
            jd = jc % ND
            wt = wqk_sb[jc]
            qps = [ps("pq", [P, 512], f"qp{jc}_{tn}") for tn in range(2)]
            for kc in range(ND):
                for tn in range(2):
                    nc.tensor.matmul(qps[tn][:], wt[:, kc, :],
                                     hT[:, kc, tn * 512:(tn + 1) * 512],
                                     start=(kc == 0), stop=(kc == ND - 1))
            for tn in range(2):
                nc.vector.tensor_copy(dst[:, jd, tn * 512:(tn + 1) * 512],
                                      qps[tn][:])

        def stats_group(which, g):
            """Batched LN stats for head group g (chunks 4g..4g+3).

            Returns A (rstd) / B (-mean*rstd) [8, T] bf16; row j = head
            8g+j.  stats psum per tn: sum rows 0:8 @0, sumsq rows 0:8 @64.
            """
            src_t = qT if which == 0 else kT
            jts = range(4 * g, 4 * g + 4)
            sqs = {}
            for jt in jts:
                sq = temps.tile([P, T], BF16, tag="sqt", bufs=2,
                                name=f"sq{which}_{jt}")
                nc.vector.tensor_mul(sq[:], src_t[:, jt, :], src_t[:, jt, :])
                sqs[jt] = sq
            A = small.tile([8, T], BF16, tag="Asb", bufs=2, name=f"A{which}_{g}")
            Bt = small.tile([8, T], BF16, tag="Bsb", bufs=2, name=f"B{which}_{g}")
            for tn in range(2):
                sl = slice(tn * 512, (tn + 1) * 512)
                stp = ps("pm", [P, 512], f"st{which}_{g}_{tn}")
                for i, jt in enumerate(jts):
                    es = eseg_sb[:, jt, 8 * g:8 * g + 8]
                    nc.tensor.matmul(stp[0:8, :], es, src_t[:, jt, sl],
                                     start=(i == 0), stop=(i == 3))
                    nc.tensor.matmul(stp[64:72, :], es, sqs[jt][:, sl],
                                     start=(i == 0), stop=(i == 3))

                def scr8(nm):
                    return small.tile([8, 512], F32, tag="scr2", bufs=4,
                                      name=f"{nm}_{which}_{g}_{tn}")

                # mu2 = (sum/E)^2 ; var = ssq/E - mu2 ; A = 1/sqrt(var+eps)
                mu = scr8("mu")
                nc.vector.tensor_scalar_mul(mu[:], stp[0:8, :], 1.0 / E)
                m2 = scr8("m2")
                nc.vector.tensor_mul(m2[:], mu[:], mu[:])
                var = scr8("var")
                nc.vector.scalar_tensor_tensor(var[:], stp[64:72, :], 1.0 / E,
                                               m2[:], OP.mult, OP.subtract)
                sd = scr8("sd")
                nc.scalar.activation(sd[:], var[:], AF.Sqrt, bias=eps_sb[0:8])
                with nc.allow_low_precision(reason="bf16 LN rstd"):
                    nc.vector.reciprocal(A[:, sl], sd[:])
                nc.vector.scalar_tensor_tensor(Bt[:, sl], mu[:], -1.0,
                                               A[:, sl], OP.mult, OP.mult)
            return A, Bt

        def affine_rope(which, jt, A, Bt):
            """src = rope(src*bcA + bcB) in place.

            bcA/bcB broadcast from [8, T] group rows via bsegJ matmul,
            consumed directly from psum; cs/sn muls on Pool engine.
            """
            src = (qT if which == 0 else kT)[:, jt, :]
            wcol = wq_sb if which == 0 else wk_sb
            bj = bsegj_sb[jt % 4]
            t2 = temps.tile([P, T], BF16, tag="ropet", bufs=2,
                            name=f"t2r{which}_{jt}")
            for tn in range(2):
                sl = slice(tn * 512, (tn + 1) * 512)
                bpA = ps("pm", [P, 512], f"bpA{which}_{jt}_{tn}")
                nc.tensor.matmul(bpA[:], bj[:], A[:, sl])
                bpB = ps("pq", [P, 512], f"bpB{which}_{jt}_{tn}")
                nc.tensor.matmul(bpB[:], bj[:], Bt[:, sl])
                nc.vector.tensor_mul(t2[:, sl], src[:, sl], bpA[:])
                nc.vector.tensor_add(t2[:, sl], t2[:, sl], bpB[:])
            if apply_qk_weight:
                nc.vector.tensor_scalar_mul(t2[:], t2[:], wcol[:])
            shf = temps.tile([P, T], BF16, tag="ropes", bufs=2,
                             name=f"shf{which}_{jt}")
            nc.vector.stream_shuffle(shf[:], t2[:], SWAP_MASK)
            nc.gpsimd.tensor_mul(src, t2[:], cs_sb[:])
            nc.gpsimd.tensor_mul(shf[:], shf[:], sn_sb[:])
            nc.vector.tensor_add(src, src, shf[:])

        def head_attention(h):
            """scores -> exp -> o accumulation -> raw oTn rows + den row."""
            jc = h // 2
            p0 = (h % 2) * E
            ops = [ps("po", [E + 16, 512], f"o{h}_{tn}") for tn in range(2)]

            def av(tk, ex):
                for tn in range(2):
                    sl = slice(tn * 512, (tn + 1) * 512)
                    nc.tensor.matmul(ops[tn][:], v_sb[:, tk, h, :], ex[:, sl],
                                     start=(tk == 0), stop=(tk == NT - 1))

            # 1-deep software pipeline: exp(tk) overlaps av(tk-1)+sc(tk+1)
            prev = None
            for tk in range(NT):
                ex = temps.tile([P, T], BF16, tag="exp", bufs=3,
                                name=f"ex{h}_{tk}")
                for tn in range(2):
                    sl = slice(tn * 512, (tn + 1) * 512)
                    sc = ps("sc", [P, 512], f"sc{h}_{tk}_{tn}")
                    nc.tensor.matmul(sc[:],
                                     kT[p0:p0 + E, jc, tk * P:(tk + 1) * P],
                                     qT[p0:p0 + E, jc, sl])
                    nc.scalar.activation(ex[:, sl], sc[:], AF.Exp, scale=0.125)
                if prev is not None:
                    av(*prev)
                prev = (tk, ex)
            av(*prev)
            # evacuate raw o rows; accumulate den (row E+h holds head h's
            # denominator, other rows zero) - normalized in the epilogue
            for tn in range(2):
                sl = slice(tn * 512, (tn + 1) * 512)
                nc.vector.tensor_copy(oTn[p0:p0 + E, jc, sl], ops[tn][0:E, :])
                nc.vector.tensor_add(denA[:, sl], denA[:, sl],
                                     ops[tn][E:E + 16, :])

        # ==============================================================
        # Pipeline: group g stats/rope/attention overlap group g+1 qk
        # ==============================================================
        denA = small.tile([16, T], F32, tag="denA", name="denA")
        nc.vector.memset(denA[:], 0.0)
        g1_jcs = [jc for jt in range(4, 8) for jc in (jt, ND + jt)]
        for jt in range(4):
            qk_chunk(jt)
            qk_chunk(ND + jt)
        for tt in range(NT):
            v_block(tt)

        wof = None
        for g in range(2):
            Aq, Bq = stats_group(0, g)
            Ak, Bk = stats_group(1, g)
            for jt in range(4 * g, 4 * g + 4):
                affine_rope(0, jt, Aq, Bq)
                affine_rope(1, jt, Ak, Bk)
            for i, h in enumerate(range(8 * g, 8 * g + 8)):
                head_attention(h)
                if g == 0:
                    # interleave group-1 projections between heads
                    qk_chunk(g1_jcs[i])
                    if i == 0:
                        for n2 in range(2):
                            mw = wbig_tile(f"modwg_{n2}")
                            col0 = 2 * D + n2 * 512
                            nc.sync.dma_start(
                                mw[:], modw_e[:, col0:col0 + 512].rearrange(
                                    "(kc p) j -> p kc j", p=P))
                            modw_sb[(2, n2)] = mw
                    if i == 2:
                        growb = mod_group(2, (modw_sb[(2, 0)], modw_sb[(2, 1)]))
                        gateB = consts.tile([P, D], BF16, tag="gateB",
                                            name="gateB")
                        for n2 in range(2):
                            sl = slice(n2 * 512, (n2 + 1) * 512)
                            bp = ps("pm", [P, 512], f"gbc{n2}")
                            nc.tensor.matmul(bp[:], ones_sb[:], growb[:, sl])
                            nc.vector.tensor_copy(gateB[:, sl], bp[:])
                    if i == 4:
                        wof = [wbig_tile(f"wof{tn}") for tn in range(2)]
                        for tn in range(2):
                            nc.sync.dma_start(
                                wof[tn][:],
                                wout_e[:, tn * 512:(tn + 1) * 512].rearrange(
                                    "(kc p) j -> p kc j", p=P))

        # ==============================================================
        # Epilogue: batched softmax normalization of oTn
        # ==============================================================
        rcpA = small.tile([16, T], BF16, tag="rcpA", name="rcpA")
        with nc.allow_low_precision(reason="bf16 softmax denominators"):
            nc.vector.reciprocal(rcpA[:], denA[:])
        for jt in range(ND):
            for tn in range(2):
                sl = slice(tn * 512, (tn + 1) * 512)
                br = ps("pm", [P, 512], f"brn{jt}_{tn}")
                nc.tensor.matmul(br[:], bseg16_sb[:, jt, :], rcpA[:, sl])
                nc.vector.tensor_mul(oTn[:, jt, sl], oTn[:, jt, sl], br[:])

        # ==============================================================
        # Stage 5: y = (oTn.T @ w_out) * gate
        # ==============================================================
        for tt in range(NT):
            y_sb = temps.tile([P, D], F32, tag="ysb", bufs=2, name=f"y{tt}")
            yps = [ps("pq", [P, 512], f"yp{tt}_{tn}") for tn in range(2)]
            for kc in range(ND):
                for tn in range(2):
                    nc.tensor.matmul(yps[tn][:], oTn[:, kc, tt * P:(tt + 1) * P],
                                     wof[tn][:, kc, :],
                                     start=(kc == 0), stop=(kc == ND - 1))
            for tn in range(2):
                sl = slice(tn * 512, (tn + 1) * 512)
                nc.vector.tensor_mul(y_sb[:, sl], yps[tn][:], gateB[:, sl])
            nc.sync.dma_start(out_e[tt * P:(tt + 1) * P, :], y_sb[:])


# =====================================================================
# Host side
# =====================================================================
_NC_CACHE = {}


def _get_nc(apply_qk_weight: bool):
    key = bool(apply_qk_weight)
    if key not in _NC_CACHE:
        _NC_CACHE[key] = build_nc(key)
    return _NC_CACHE[key]


def _make_consts(position, q_norm_w, k_norm_w):
    cs = np.ones((P, T), np.float32)
    sn = np.zeros((P, T), np.float32)
    cos = position[:, :, 0].T.astype(np.float32)   # [16, T]
    sin = position[:, :, 1].T.astype(np.float32)
    for half in (0, 64):
        for rr in range(32):
            j = rr // 2
            cs[half + rr, :] = cos[j]
            sn[half + rr, :] = sin[j] if (rr % 2 == 1) else -sin[j]
    eseg = np.zeros((P, ND, 16), np.float32)
    bseg16 = np.zeros((16, ND, P), np.float32)
    for t in range(ND):
        for p in range(P):
            m = 2 * t + p // E
            eseg[p, t, m] = 1.0
            bseg16[m, t, p] = 1.0
    bsegj = np.zeros((8, 4, P), np.float32)
    for j in range(4):
        for p in range(P):
            bsegj[2 * j + p // E, j, p] = 1.0
    import ml_dtypes  # noqa: deferred import keeps numpy-only callers fast
    return dict(
        cs_full=cs.astype(ml_dtypes.bfloat16), sn_full=sn.astype(ml_dtypes.bfloat16),
        eseg=eseg.astype(ml_dtypes.bfloat16),
        bsegj=bsegj.astype(ml_dtypes.bfloat16),
        bseg16=bseg16.astype(ml_dtypes.bfloat16),
        ident=np.eye(P, dtype=np.float32).astype(ml_dtypes.bfloat16),
        ones_row=np.ones((1, P), np.float32).astype(ml_dtypes.bfloat16),
        wq_col=np.tile(q_norm_w.astype(np.float32), 2).reshape(P, 1),
        wk_col=np.tile(k_norm_w.astype(np.float32), 2).reshape(P, 1),
    )


def _prep_weights(mod_w, w_qkv, w_out):
    import ml_dtypes
    return dict(
        mod_w=np.ascontiguousarray(np.asarray(mod_w, np.float32)
                                   .astype(ml_dtypes.bfloat16)),
        w_qkv=np.ascontiguousarray(np.asarray(w_qkv, np.float32)
                                   .astype(ml_dtypes.bfloat16)),
        w_out=np.ascontiguousarray(np.asarray(w_out, np.float32)
                                   .astype(ml_dtypes.bfloat16)),
    )


def kernel(x, time, position, mod_w, mod_b, w_qkv, w_out, q_norm_w, k_norm_w):
    x = np.ascontiguousarray(np.asarray(x, dtype=np.float32))
    time = np.ascontiguousarray(np.asarray(time, dtype=np.float32))
    position = np.asarray(position, dtype=np.float32)
    mod_b = np.ascontiguousarray(np.asarray(mod_b, dtype=np.float32))
    q_norm_w = np.asarray(q_norm_w, dtype=np.float32)
    k_norm_w = np.asarray(k_norm_w, dtype=np.float32)
    wts = _prep_weights(mod_w, w_qkv, w_out)

    apply_w = not (np.all(q_norm_w == 1.0) and np.all(k_norm_w == 1.0))
    nc = _get_nc(apply_w)
    consts = _make_consts(position, q_norm_w, k_norm_w)

    in_maps = [
        dict(x=x[b], time=time[b].reshape(TD), mod_b=mod_b, **wts, **consts)
        for b in range(B)
    ]
    res = run_bass_kernel_spmd(nc, in_maps, core_ids=list(range(B)))
    out = np.stack([res.results[b]["out"] for b in range(B)], axis=0)
    return out.astype(np.float32)


if __name__ == "__main__":
    nc = build_nc(False)
    print("graph built ok")


# revision 46
# speedup vs baseline: 1.4948x; 1.0001x over previous
"""AdaLN-modulated multi-head attention block on 8 TRN2 NeuronCores.

Shapes (hardcoded): B=8, T=1024, D=1024, H=16 heads, e=64 head dim.
Sharding: pure data-parallel - one batch element per core, weights
replicated, no collectives.

v2 design (vs baseline):
  - f32r moving/stationary weights: no f32->bf16 conversion pass.
  - per-head-pair pipeline: qk projection chunk -> stats -> affine+rope
    -> scores/exp/av per head, so ACT exp overlaps all PE work.
  - RoPE pair swap via DVE stream_shuffle (not PE matmul).
  - rope/affine elementwise all-bf16-SBUF for DVE 2x/4x perf modes.
  - psum evacuations distributed to Pool (gpsimd) / ACT.
  - bulk rearranged DMAs; gate + w_out loads deferred to mid-kernel.

Per-core pipeline ("T" suffix = feature-major [feature, token] layout):
  0. mod = silu(time) @ mod_w + mod_b (f32r moving); shift/scale cols.
  1. h = LN(x) (ACT apply) -> PE transpose -> fused affine evac -> hT
  2. v = h @ w_qkv[:, 2048:] (f32r moving, + ones col per head)
  3. per hp in 0..7:  qT/kT chunk (w stationary f32r),
     per-chunk LN stats (eseg matmul), affine+rope (DVE bf16),
     per head: scoresT = kT.T@qT; exp (ACT); oT = [v|1].T@exp;
     normalize via den-broadcast matmul + fused psum mul evac.
  4. y = (oT.T @ w_out) * gate (f32r moving w_out)
"""

import sys

try:
    import concourse  # noqa: F401  (provided by the environment, e.g. axon_site)
except ImportError:
    sys.path.append("/opt/trn_rl_repo")

import contextlib

import numpy as np

import concourse.bass as bass
import concourse.mybir as mybir
import concourse.tile as tile
from concourse import bacc
from concourse.bass_utils import run_bass_kernel_spmd

F32 = mybir.dt.float32
F32R = mybir.dt.float32r
BF16 = mybir.dt.bfloat16
AF = mybir.ActivationFunctionType
OP = mybir.AluOpType

B, T, D, TD = 8, 1024, 1024, 1024
H, E = 16, 64
P = 128
NT = T // P          # 8 token tiles
ND = D // P          # 8 feature tiles
EPS = 1e-6
N3 = 3 * D
SWAP_MASK = [i ^ 1 for i in range(32)]


def r(ap):
    """View an f32 AP as float32r for full-rate TensorE matmuls."""
    return ap.bitcast(F32R)


def build_nc(apply_qk_weight: bool):
    nc = bacc.Bacc("TRN2", target_bir_lowering=False, debug=False, num_devices=8)

    aps = {}
    aps["x"] = nc.dram_tensor("x", [T, D], F32, kind="ExternalInput").ap()
    aps["time"] = nc.dram_tensor("time", [TD], F32, kind="ExternalInput").ap()
    # weights arrive host-preconverted to bf16 (halves DMA, no on-chip casts)
    aps["mod_w"] = nc.dram_tensor("mod_w", [TD, N3], BF16, kind="ExternalInput").ap()
    aps["mod_b"] = nc.dram_tensor("mod_b", [N3], F32, kind="ExternalInput").ap()
    aps["w_qkv"] = nc.dram_tensor("w_qkv", [D, N3], BF16, kind="ExternalInput").ap()
    aps["w_out"] = nc.dram_tensor("w_out", [D, D], BF16, kind="ExternalInput").ap()
    # host-precomputed constants
    aps["cs_full"] = nc.dram_tensor("cs_full", [P, T], BF16, kind="ExternalInput").ap()
    aps["sn_full"] = nc.dram_tensor("sn_full", [P, T], BF16, kind="ExternalInput").ap()
    aps["eseg"] = nc.dram_tensor("eseg", [P, ND, 16], BF16, kind="ExternalInput").ap()
    aps["bsegj"] = nc.dram_tensor("bsegj", [8, 4, P], BF16, kind="ExternalInput").ap()
    aps["bseg16"] = nc.dram_tensor("bseg16", [16, ND, P], BF16,
                                   kind="ExternalInput").ap()
    aps["ident"] = nc.dram_tensor("ident", [P, P], BF16, kind="ExternalInput").ap()
    aps["ones_row"] = nc.dram_tensor("ones_row", [1, P], BF16, kind="ExternalInput").ap()
    aps["wq_col"] = nc.dram_tensor("wq_col", [P, 1], F32, kind="ExternalInput").ap()
    aps["wk_col"] = nc.dram_tensor("wk_col", [P, 1], F32, kind="ExternalInput").ap()

    aps["out"] = nc.dram_tensor("out", [T, D], F32, kind="ExternalOutput").ap()

    with tile.TileContext(nc) as tc:
        _body(nc, tc, aps, apply_qk_weight)
    nc.finalize()
    return nc


def _body(nc, tc, aps, apply_qk_weight):
    x_e, time_e, modw_e = aps["x"], aps["time"], aps["mod_w"]
    modb_e, wqkv_e, wout_e = aps["mod_b"], aps["w_qkv"], aps["w_out"]
    out_e = aps["out"]

    ctx = contextlib.ExitStack()
    with ctx:
        consts = ctx.enter_context(tc.tile_pool(name="consts", bufs=1))
        big = ctx.enter_context(tc.tile_pool(name="big", bufs=1))
        wstr = ctx.enter_context(tc.tile_pool(name="wstr", bufs=1))
        temps = ctx.enter_context(tc.tile_pool(name="temps", bufs=2))
        small = ctx.enter_context(tc.tile_pool(name="small", bufs=1))
        psum = ctx.enter_context(tc.tile_pool(name="psum", bufs=2, space="PSUM"))

        # PSUM budget (8 banks of [128 x 2KB]):
        #   sc: 2 bufs [P,512]  - score halves
        #   po: 2 bufs [P,512]  - o accumulation (per head: tn0,tn1)
        #   pq: 2 bufs [P,512]  - q/k projection chains
        #   pm: 2 bufs [P,512]  - misc (stats, bcasts, mod, transposes, y)
        def ps(tag, shape, name, dtype=F32):
            return psum.tile(shape, dtype, tag=tag, bufs=2, name=name,
                             padded_shape=[P, 512])

        # ---- constants into SBUF -------------------------------------
        def cload(key, shape, dtype, name):
            t = consts.tile(shape, dtype, tag=name, name=name)
            nc.sync.dma_start(t[:], aps[key])
            return t

        cs_sb = cload("cs_full", [P, T], BF16, "cs_sb")
        sn_sb = cload("sn_full", [P, T], BF16, "sn_sb")
        eseg_sb = cload("eseg", [P, ND, 16], BF16, "eseg_sb")
        bsegj_all = cload("bsegj", [8, 4, P], BF16, "bsegj_sb")
        bsegj_sb = [bsegj_all[:, j, :] for j in range(4)]
        bseg16_sb = cload("bseg16", [16, ND, P], BF16, "bseg16_sb")
        ident_sb = cload("ident", [P, P], BF16, "ident_sb")
        ones_sb = cload("ones_row", [1, P], BF16, "ones_sb")
        wq_sb = cload("wq_col", [P, 1], F32, "wq_sb")
        wk_sb = cload("wk_col", [P, 1], F32, "wk_sb")
        eps_sb = consts.tile([P, 1], F32, tag="eps_sb", name="eps_sb")
        nc.gpsimd.memset(eps_sb[:], EPS)

        # ---- big resident tensors ------------------------------------
        hT = big.tile([P, ND, T], BF16, tag="hT", name="hT")       # 16K/part
        qT = big.tile([P, ND, T], BF16, tag="qT", name="qT")       # 16K
        kT = big.tile([P, ND, T], BF16, tag="kT", name="kT")       # 16K
        v_sb = big.tile([P, NT, H, E + 8], BF16, tag="v", name="v_sb")  # 18K
        oTn = qT   # head rows of qT are dead once that head's scores ran

        # ==============================================================
        # DMA prefetch order: modw(shift+scale) -> x -> wqk -> wv ->
        # wout -> modw(gate).  Issue the early ones now.
        # ==============================================================
        # Shared [128, 8kc, 512] bf16 staging ring for mod_w / w_qkv-v /
        # w_out blocks (all the same shape; ring order = lifetime order).
        def wbig_tile(name):
            return wstr.tile([P, ND, 512], BF16, tag="wbig", bufs=3, name=name)

        # mod_w shift+scale halves first
        modw_sb = {}
        for g, n2 in ((0, 0), (0, 1), (1, 0), (1, 1)):
            mw = wbig_tile(f"modw{g}_{n2}")
            col0 = g * D + n2 * 512
            nc.sync.dma_start(
                mw[:], modw_e[:, col0:col0 + 512].rearrange(
                    "(kc p) j -> p kc j", p=P))
            modw_sb[(g, n2)] = mw

        # x tiles
        x_tiles = []
        for tt in range(NT):
            xt = temps.tile([P, D], F32, tag="xt", bufs=2, name=f"xt{tt}")
            nc.sync.dma_start(xt[:], x_e[tt * P:(tt + 1) * P, :])
            x_tiles.append(xt)

        # w_qkv q/k column blocks, one [128, 8kc, 128] tile per jc,
        # ordered by head pair: hp, 8+hp
        wqk_sb = {}
        for hp in range(ND):
            for jc in (hp, ND + hp):
                wt = wstr.tile([P, ND, P], BF16, tag="wqk", bufs=6,
                               name=f"wqk{jc}")
                nc.sync.dma_start(
                    wt[:], wqkv_e[:, jc * P:(jc + 1) * P].rearrange(
                        "(kc p) j -> p kc j", p=P))
                wqk_sb[jc] = wt

        # w_qkv v blocks: [128, 8kc, 512] per nv (wbig ring, after mod)
        wv_sb = []
        for nv in range(2):
            wv = wbig_tile(f"wv{nv}")
            nc.sync.dma_start(
                wv[:], wqkv_e[:, 2048 + nv * 512:2048 + (nv + 1) * 512]
                .rearrange("(kc p) j -> p kc j", p=P))
            wv_sb.append(wv)

        # ==============================================================
        # Stage 0: mod = silu(time) @ mod_w + mod_b (shift/scale now)
        # ==============================================================
        t8 = small.tile([P, TD // P], F32, tag="t8", name="t8")
        nc.sync.dma_start(t8[:], time_e.rearrange("(o p) -> p o", p=P))
        silu8 = small.tile([P, TD // P], BF16, tag="silu8", name="silu8")
        nc.scalar.activation(silu8[:], t8[:], AF.Silu)

        def mod_group(g, mw_pair):
            """Compute mod group g -> [1, D] bf16 row (bias added)."""
            mrowf = temps.tile([1, D], F32, tag="rbc", bufs=1, name=f"mrowf{g}")
            nc.sync.dma_start(mrowf[:], modb_e[None, g * D:(g + 1) * D])
            mrow = small.tile([1, D], BF16, tag=f"mrow{g}", bufs=1,
                              name=f"mrow{g}")
            for n2 in range(2):
                mw = mw_pair[n2]
                mp = ps("pm", [1, 512], f"modp{g}_{n2}")
                for kc in range(TD // P):
                    nc.tensor.matmul(mp[:], silu8[:, kc:kc + 1], mw[:, kc, :],
                                     start=(kc == 0), stop=(kc == TD // P - 1))
                sl = slice(n2 * 512, (n2 + 1) * 512)
                nc.vector.tensor_add(mrow[:, sl], mrowf[:, sl], mp[:])
            return mrow

        def mod_to_cols(g, mrow, plus1):
            """Transpose a [1, D] mod row into per-feature cols [P, ND]."""
            if plus1:
                nc.vector.tensor_scalar_add(mrow[:], mrow[:], 1.0)
            col = consts.tile([P, ND], F32, tag=f"col{g}", name=f"col{g}")
            cp = psum.tile([P, 2 * ND], BF16, tag="pm", bufs=2,
                           name=f"colp{g}", padded_shape=[P, 512])
            for dc in range(ND):
                nc.tensor.transpose(cp[:, 2 * dc:2 * dc + 1],
                                    mrow[:, dc * P:(dc + 1) * P],
                                    ident_sb[0:1, 0:1])
            nc.vector.tensor_copy(col[:], cp[:].rearrange(
                "p (d two) -> p d two", two=2)[:, :, 0])
            return col

        shcol = mod_to_cols(0, mod_group(0, (modw_sb[(0, 0)], modw_sb[(0, 1)])),
                            False)
        sc1col = mod_to_cols(1, mod_group(1, (modw_sb[(1, 0)], modw_sb[(1, 1)])),
                             True)

        # ==============================================================
        # Stage 1: h = LN(x)*(scale+1)+shift ; hT = h.T
        # ==============================================================
        for tt in range(NT):
            xt = x_tiles[tt]
            st = small.tile([P, 2, 6], F32, tag="bnst", bufs=2, name=f"st{tt}")
            nc.vector.bn_stats(st[:, 0, :], xt[:, 0:512])
            nc.vector.bn_stats(st[:, 1, :], xt[:, 512:1024])
            mv = small.tile([P, 2], F32, tag="bnmv", bufs=2, name=f"mv{tt}")
            nc.vector.bn_aggr(mv[:], st[:])
            sd = small.tile([P, 1], F32, tag="sd", bufs=2, name=f"sd{tt}")
            nc.scalar.activation(sd[:], mv[:, 1:2], AF.Sqrt, bias=eps_sb[:])
            rstd = small.tile([P, 1], F32, tag="rstd", bufs=2, name=f"rstd{tt}")
            nc.vector.reciprocal(rstd[:], sd[:])
            nmr = small.tile([P, 1], F32, tag="nmr", bufs=2, name=f"nmr{tt}")
            nc.vector.tensor_scalar(nmr[:], mv[:, 0:1], rstd[:], -1.0,
                                    OP.mult, OP.mult)
            xn = temps.tile([P, D], BF16, tag="xnb", bufs=2, name=f"xn{tt}")
            nc.scalar.activation(xn[:], xt[:], AF.Identity, bias=nmr[:],
                                 scale=rstd[:])
            # 4 transposes per [P,512] bf16 psum buf, fused affine evac
            for half in range(2):
                tp = psum.tile([P, 512], BF16, tag="pq", bufs=2,
                               name=f"tr{tt}_{half}", padded_shape=[P, 512])
                for q in range(4):
                    dc = half * 4 + q
                    nc.tensor.transpose(tp[:, q * P:(q + 1) * P],
                                        xn[:, dc * P:(dc + 1) * P], ident_sb[:])
                for q in range(4):
                    dc = half * 4 + q
                    if dc % 2 == 0:
                        nc.vector.tensor_scalar(
                            hT[:, dc, tt * P:(tt + 1) * P],
                            tp[:, q * P:(q + 1) * P],
                            sc1col[:, dc:dc + 1], shcol[:, dc:dc + 1],
                            OP.mult, OP.add)
                    else:
                        nc.scalar.activation(
                            hT[:, dc, tt * P:(tt + 1) * P],
                            tp[:, q * P:(q + 1) * P], AF.Identity,
                            bias=shcol[:, dc:dc + 1],
                            scale=sc1col[:, dc:dc + 1])

        # ==============================================================
        # Stage 2a: v = h @ w_qkv[:, 2048:] (+ ones col per head)
        # ==============================================================
        nc.gpsimd.memset(v_sb[:, :, :, E:E + 8], 0.0)
        for h in range(H):
            nc.gpsimd.memset(v_sb[:, :, h, E + h % 8], 1.0)

        def v_block(tt):
            vps = [ps("po", [P, 512], f"vp{tt}_{nv}") for nv in range(2)]
            for kc in range(ND):
                for nv in range(2):
                    nc.tensor.matmul(vps[nv][:], hT[:, kc, tt * P:(tt + 1) * P],
                                     wv_sb[nv][:, kc, :],
                                     start=(kc == 0), stop=(kc == ND - 1))
            for nv in range(2):
                nc.scalar.copy(
                    v_sb[:, tt, nv * 8:(nv + 1) * 8, 0:E],
                    vps[nv][:].rearrange("p (h e) -> p h e", e=E))

        # ==============================================================
        # Building blocks
        # ==============================================================
        def qk_chunk(jc):
            """Project w_qkv col block jc against hT -> qT/kT chunk."""
            dst = qT if jc < ND else kT
            jd = jc % ND
            wt = wqk_sb[jc]
            qps = [ps("pq", [P, 512], f"qp{jc}_{tn}") for tn in range(2)]
            for kc in range(ND):
                for tn in range(2):
                    nc.tensor.matmul(qps[tn][:], wt[:, kc, :],
                                     hT[:, kc, tn * 512:(tn + 1) * 512],
                                     start=(kc == 0), stop=(kc == ND - 1))
            for tn in range(2):
                nc.vector.tensor_copy(dst[:, jd, tn * 512:(tn + 1) * 512],
                                      qps[tn][:])

        def stats_group(which, g):
            """Batched LN stats for head group g (chunks 4g..4g+3).

            Returns A (rstd) / B (-mean*rstd) [8, T] bf16; row j = head
            8g+j.  stats psum per tn: sum rows 0:8 @0, sumsq rows 0:8 @64.
            """
            src_t = qT if which == 0 else kT
            jts = range(4 * g, 4 * g + 4)
            sqs = {}
            for jt in jts:
                sq = temps.tile([P, T], BF16, tag="sqt", bufs=2,
                                name=f"sq{which}_{jt}")
                nc.vector.tensor_mul(sq[:], src_t[:, jt, :], src_t[:, jt, :])
                sqs[jt] = sq
            A = small.tile([8, T], BF16, tag="Asb", bufs=2, name=f"A{which}_{g}")
            Bt = small.tile([8, T], BF16, tag="Bsb", bufs=2, name=f"B{which}_{g}")
            for tn in range(2):
                sl = slice(tn * 512, (tn + 1) * 512)
                stp = ps("pm", [P, 512], f"st{which}_{g}_{tn}")
                for i, jt in enumerate(jts):
                    es = eseg_sb[:, jt, 8 * g:8 * g + 8]
                    nc.tensor.matmul(stp[0:8, :], es, src_t[:, jt, sl],
                                     start=(i == 0), stop=(i == 3))
                    nc.tensor.matmul(stp[64:72, :], es, sqs[jt][:, sl],
                                     start=(i == 0), stop=(i == 3))

                def scr8(nm):
                    return small.tile([8, 512], F32, tag="scr2", bufs=4,
                                      name=f"{nm}_{which}_{g}_{tn}")

                # mu2 = (sum/E)^2 ; var = ssq/E - mu2 ; A = 1/sqrt(var+eps)
                mu = scr8("mu")
                nc.vector.tensor_scalar_mul(mu[:], stp[0:8, :], 1.0 / E)
                m2 = scr8("m2")
                nc.vector.tensor_mul(m2[:], mu[:], mu[:])
                var = scr8("var")
                nc.vector.scalar_tensor_tensor(var[:], stp[64:72, :], 1.0 / E,
                                               m2[:], OP.mult, OP.subtract)
                sd = scr8("sd")
                nc.scalar.activation(sd[:], var[:], AF.Sqrt, bias=eps_sb[0:8])
                with nc.allow_low_precision(reason="bf16 LN rstd"):
                    nc.vector.reciprocal(A[:, sl], sd[:])
                nc.vector.scalar_tensor_tensor(Bt[:, sl], mu[:], -1.0,
                                               A[:, sl], OP.mult, OP.mult)
            return A, Bt

        def affine_rope(which, jt, A, Bt):
            """src = rope(src*bcA + bcB) in place.

            bcA/bcB broadcast from [8, T] group rows via bsegJ matmul,
            consumed directly from psum; cs/sn muls on Pool engine.
            """
            src = (qT if which == 0 else kT)[:, jt, :]
            wcol = wq_sb if which == 0 else wk_sb
            bj = bsegj_sb[jt % 4]
            t2 = temps.tile([P, T], BF16, tag="ropet", bufs=2,
                            name=f"t2r{which}_{jt}")
            for tn in range(2):
                sl = slice(tn * 512, (tn + 1) * 512)
                bpA = ps("pm", [P, 512], f"bpA{which}_{jt}_{tn}")
                nc.tensor.matmul(bpA[:], bj[:], A[:, sl])
                bpB = ps("pq", [P, 512], f"bpB{which}_{jt}_{tn}")
                nc.tensor.matmul(bpB[:], bj[:], Bt[:, sl])
                nc.vector.tensor_mul(t2[:, sl], src[:, sl], bpA[:])
                nc.vector.tensor_add(t2[:, sl], t2[:, sl], bpB[:])
            if apply_qk_weight:
                nc.vector.tensor_scalar_mul(t2[:], t2[:], wcol[:])
            shf = temps.tile([P, T], BF16, tag="ropes", bufs=2,
                             name=f"shf{which}_{jt}")
            nc.vector.stream_shuffle(shf[:], t2[:], SWAP_MASK)
            nc.gpsimd.tensor_mul(src, t2[:], cs_sb[:])
            nc.gpsimd.tensor_mul(shf[:], shf[:], sn_sb[:])
            nc.vector.tensor_add(src, src, shf[:])

        def head_attention(h):
            """scores -> exp -> o accumulation -> raw oTn rows + den row."""
            jc = h // 2
            p0 = (h % 2) * E
            ops = [ps("po", [E + 8, 512], f"o{h}_{tn}") for tn in range(2)]

            def av(tk, ex):
                for tn in range(2):
                    sl = slice(tn * 512, (tn + 1) * 512)
                    nc.tensor.matmul(ops[tn][:], v_sb[:, tk, h, :], ex[:, sl],
                                     start=(tk == 0), stop=(tk == NT - 1))

            # 1-deep software pipeline: exp(tk) overlaps av(tk-1)+sc(tk+1)
            prev = None
            for tk in range(NT):
                ex = temps.tile([P, T], BF16, tag="exp", bufs=3,
                                name=f"ex{h}_{tk}")
                for tn in range(2):
                    sl = slice(tn * 512, (tn + 1) * 512)
                    sc = ps("sc", [P, 512], f"sc{h}_{tk}_{tn}")
                    nc.tensor.matmul(sc[:],
                                     kT[p0:p0 + E, jc, tk * P:(tk + 1) * P],
                                     qT[p0:p0 + E, jc, sl])
                    nc.scalar.activation(ex[:, sl], sc[:], AF.Exp, scale=0.125)
                if prev is not None:
                    av(*prev)
                prev = (tk, ex)
            av(*prev)
            # evacuate raw o rows; accumulate den (row E+h%8 holds head
            # h's denominator, other rows zero) - normalized later
            deng = denA[h // 8]
            for tn in range(2):
                sl = slice(tn * 512, (tn + 1) * 512)
                nc.vector.tensor_copy(oTn[p0:p0 + E, jc, sl], ops[tn][0:E, :])
                nc.vector.tensor_add(deng[:, sl], deng[:, sl],
                                     ops[tn][E:E + 8, :])

        # ==============================================================
        # Pipeline: rope/stats/projections interleave with attention so
        # the PE never idles (idle PE triggers the k=4 half-clock state)
        # ==============================================================
        denA = [small.tile([8, T], F32, tag=f"denA{g}", name=f"denA{g}")
                for g in range(2)]
        for g in range(2):
            nc.vector.memset(denA[g][:], 0.0)
        g1_jcs = [jc for jt in range(4, 8) for jc in (jt, ND + jt)]
        for jt in range(4):
            qk_chunk(jt)
            qk_chunk(ND + jt)
        for tt in range(NT):
            v_block(tt)
        Aq0, Bq0 = stats_group(0, 0)
        Ak0, Bk0 = stats_group(1, 0)
        affine_rope(0, 0, Aq0, Bq0)
        affine_rope(1, 0, Ak0, Bk0)

        rcp8 = [small.tile([8, T], BF16, tag=f"rcp8_{g}", name=f"rcp8_{g}")
                for g in range(2)]

        def normalize_chunk(jt, rcp):
            bj = bsegj_sb[jt % 4]
            for tn in range(2):
                sl = slice(tn * 512, (tn + 1) * 512)
                br = ps("pm", [P, 512], f"brn{jt}_{tn}")
                nc.tensor.matmul(br[:], bj[:], rcp[:, sl])
                nc.vector.tensor_mul(oTn[:, jt, sl], oTn[:, jt, sl], br[:])

        # ---- group 0: heads 0..7 ------------------------------------
        for jt in range(4):
            head_attention(2 * jt)
            for jc in g1_jcs[2 * jt:2 * jt + 2]:
                qk_chunk(jc)
            head_attention(2 * jt + 1)
            if jt < 3:
                affine_rope(0, jt + 1, Aq0, Bq0)
                affine_rope(1, jt + 1, Ak0, Bk0)
            if jt == 0:
                for n2 in range(2):
                    mw = wbig_tile(f"modwg_{n2}")
                    col0 = 2 * D + n2 * 512
                    nc.sync.dma_start(
                        mw[:], modw_e[:, col0:col0 + 512].rearrange(
                            "(kc p) j -> p kc j", p=P))
                    modw_sb[(2, n2)] = mw
        # group-1 stats/first rope (all 8 g1 chunks emitted above)
        Aq1, Bq1 = stats_group(0, 1)
        Ak1, Bk1 = stats_group(1, 1)
        affine_rope(0, 4, Aq1, Bq1)
        affine_rope(1, 4, Ak1, Bk1)
        # group-0 softmax normalization overlaps group-1 attention
        with nc.allow_low_precision(reason="bf16 softmax denominators"):
            nc.vector.reciprocal(rcp8[0][:], denA[0][:])
        for jt in range(4):
            normalize_chunk(jt, rcp8[0])

        # ---- group 1: heads 8..15 -----------------------------------
        wof = None
        for jt in range(4, 8):
            head_attention(2 * jt)
            if jt == 4:
                wof = [wbig_tile(f"wof{tn}") for tn in range(2)]
                for tn in range(2):
                    nc.sync.dma_start(
                        wof[tn][:],
                        wout_e[:, tn * 512:(tn + 1) * 512].rearrange(
                            "(kc p) j -> p kc j", p=P))
            if jt == 5:
                growb = mod_group(2, (modw_sb[(2, 0)], modw_sb[(2, 1)]))
                gateB = consts.tile([P, D], BF16, tag="gateB", name="gateB")
                for n2 in range(2):
                    sl = slice(n2 * 512, (n2 + 1) * 512)
                    bp = ps("pm", [P, 512], f"gbc{n2}")
                    nc.tensor.matmul(bp[:], ones_sb[:], growb[:, sl])
                    nc.vector.tensor_copy(gateB[:, sl], bp[:])
            head_attention(2 * jt + 1)
            if jt < 7:
                affine_rope(0, jt + 1, Aq1, Bq1)
                affine_rope(1, jt + 1, Ak1, Bk1)

        # ==============================================================
        # Epilogue: group-1 normalization
        # ==============================================================
        with nc.allow_low_precision(reason="bf16 softmax denominators"):
            nc.vector.reciprocal(rcp8[1][:], denA[1][:])
        for jt in range(4, 8):
            normalize_chunk(jt, rcp8[1])

        # ==============================================================
        # Stage 5: y = (oTn.T @ w_out) * gate
        # ==============================================================
        for tt in range(NT):
            y_sb = temps.tile([P, D], F32, tag="ysb", bufs=2, name=f"y{tt}")
            yps = [ps("pq", [P, 512], f"yp{tt}_{tn}") for tn in range(2)]
            for kc in range(ND):
                for tn in range(2):
                    nc.tensor.matmul(yps[tn][:], oTn[:, kc, tt * P:(tt + 1) * P],
                                     wof[tn][:, kc, :],
                                     start=(kc == 0), stop=(kc == ND - 1))
            for tn in range(2):
                sl = slice(tn * 512, (tn + 1) * 512)
                nc.vector.tensor_mul(y_sb[:, sl], yps[tn][:], gateB[:, sl])
            nc.sync.dma_start(out_e[tt * P:(tt + 1) * P, :], y_sb[:])


# =====================================================================
# Host side
# =====================================================================
_NC_CACHE = {}


def _get_nc(apply_qk_weight: bool):
    key = bool(apply_qk_weight)
    if key not in _NC_CACHE:
        _NC_CACHE[key] = build_nc(key)
    return _NC_CACHE[key]


def _make_consts(position, q_norm_w, k_norm_w):
    cs = np.ones((P, T), np.float32)
    sn = np.zeros((P, T), np.float32)
    cos = position[:, :, 0].T.astype(np.float32)   # [16, T]
    sin = position[:, :, 1].T.astype(np.float32)
    for half in (0, 64):
        for rr in range(32):
            j = rr // 2
            cs[half + rr, :] = cos[j]
            sn[half + rr, :] = sin[j] if (rr % 2 == 1) else -sin[j]
    eseg = np.zeros((P, ND, 16), np.float32)
    bseg16 = np.zeros((16, ND, P), np.float32)
    for t in range(ND):
        for p in range(P):
            m = 2 * t + p // E
            eseg[p, t, m] = 1.0
            bseg16[m, t, p] = 1.0
    bsegj = np.zeros((8, 4, P), np.float32)
    for j in range(4):
        for p in range(P):
            bsegj[2 * j + p // E, j, p] = 1.0
    import ml_dtypes  # noqa: deferred import keeps numpy-only callers fast
    return dict(
        cs_full=cs.astype(ml_dtypes.bfloat16), sn_full=sn.astype(ml_dtypes.bfloat16),
        eseg=eseg.astype(ml_dtypes.bfloat16),
        bsegj=bsegj.astype(ml_dtypes.bfloat16),
        bseg16=bseg16.astype(ml_dtypes.bfloat16),
        ident=np.eye(P, dtype=np.float32).astype(ml_dtypes.bfloat16),
        ones_row=np.ones((1, P), np.float32).astype(ml_dtypes.bfloat16),
        wq_col=np.tile(q_norm_w.astype(np.float32), 2).reshape(P, 1),
        wk_col=np.tile(k_norm_w.astype(np.float32), 2).reshape(P, 1),
    )


def _prep_weights(mod_w, w_qkv, w_out):
    import ml_dtypes
    return dict(
        mod_w=np.ascontiguousarray(np.asarray(mod_w, np.float32)
                                   .astype(ml_dtypes.bfloat16)),
        w_qkv=np.ascontiguousarray(np.asarray(w_qkv, np.float32)
                                   .astype(ml_dtypes.bfloat16)),
        w_out=np.ascontiguousarray(np.asarray(w_out, np.float32)
                                   .astype(ml_dtypes.bfloat16)),
    )


def kernel(x, time, position, mod_w, mod_b, w_qkv, w_out, q_norm_w, k_norm_w):
    x = np.ascontiguousarray(np.asarray(x, dtype=np.float32))
    time = np.ascontiguousarray(np.asarray(time, dtype=np.float32))
    position = np.asarray(position, dtype=np.float32)
    mod_b = np.ascontiguousarray(np.asarray(mod_b, dtype=np.float32))
    q_norm_w = np.asarray(q_norm_w, dtype=np.float32)
    k_norm_w = np.asarray(k_norm_w, dtype=np.float32)
    wts = _prep_weights(mod_w, w_qkv, w_out)

    apply_w = not (np.all(q_norm_w == 1.0) and np.all(k_norm_w == 1.0))
    nc = _get_nc(apply_w)
    consts = _make_consts(position, q_norm_w, k_norm_w)

    in_maps = [
        dict(x=x[b], time=time[b].reshape(TD), mod_b=mod_b, **wts, **consts)
        for b in range(B)
    ]
    res = run_bass_kernel_spmd(nc, in_maps, core_ids=list(range(B)))
    out = np.stack([res.results[b]["out"] for b in range(B)], axis=0)
    return out.astype(np.float32)


if __name__ == "__main__":
    nc = build_nc(False)
    print("graph built ok")
